# revision 21
# baseline (speedup 1.0000x reference)
"""Trainium2 Bass kernel for an ExponentialRNN (modrelu recurrence).

Computation (per example b):
    xT = x @ T                                   # [B, S, U] pre-projection
    h_{t+1} = modrelu(xT[:, t] + h_t @ B, bias)  # 512 sequential steps
    out[t] = h_{t+1}                             # [S, B, U]

Sharding: data-parallel over batch across 8 cores (8 examples/core).

Numerics: single-pass f32r matmuls with a 4-phase *dithered* rounding of B
(B_k = f32r(k*B - sum_{i<k} B_i), cycled per step) so the coherent
B-rounding drift largely cancels; state stored f32r. The recurrence is
D-scaled (w = z/|bias|, B'' = D B D^-1) so modrelu becomes
w + clamp(m*w, -1, 1) with m in {BIG, -1} per unit, and units are permuted
so each partition's chunk-pair holds same-class units -> ONE fused DVE op
per chunk pair. Host un-scales/un-permutes the output. Measured end-to-end
rel-err ~6.8e-3 (harness gate 2e-2); exact hi/lo would cost 2x matmul time.

Per-core device program, per step t (col halves H of the 512 units):
  PE : 8 accumulating f32r matmuls   zH[0:8, :] += h_k^T @ B''_k[:, H]
  DVE: copy zH  PSUM -> SBUF (zb)    [one op per half]
  PE : 4 transpose-mode matmuls  zb[8,128] -> zt-pair[128,8] (unit-major)
  DVE: fused modrelu per chunk-pair  ybuf[:, 8t] = w + clamp(m*w, -1, 1)
       reading zt (PSUM) + xT slice, writing the f32r state/archive.
Separate PSUM tiles per half (zA/zB) and per chunk-pair (zt01/zt23) keep
the coarse PSUM reader-dependencies chunk-granular, so step t+1's matmuls
start as each pair's modrelu lands (the critical cycle is
psum-B -> copy -> PT -> mod23 -> k2/k3 matmuls, ~1.9us/step).
The next step's matmul stationaries read ybuf directly. Phase 1 (xT
pre-projection, f32r, ACT-engine copies) is emitted interleaved, 1/4 block
per 16 steps, into a 4-slot ring; output DMA streams ybuf every 64 steps.
"""

import os
import sys

import numpy as np

for _p in ("/opt/trn_rl_repo", "/root/.axon_site/_ro/trn_rl_repo"):
    if os.path.isdir(_p) and _p not in sys.path:
        sys.path.insert(0, _p)

import concourse.bass as bass
import concourse.bacc as bacc
import concourse.mybir as mybir
import concourse.bass_utils as bass_utils
import concourse.dve_ops as dve_ops
from concourse.dve_spec import Spec, Src0, Src1, C0, C1, Zero, maxx, minn, lower
from concourse.dve_uop import DveOpSpec
from concourse.tile import TileContext

BATCH, SEQ, DIN, UNITS = 64, 512, 256, 512
NCORES = 8
BS = BATCH // NCORES          # per-core batch = 8
NK = UNITS // 128             # 4 unit chunks
ND = DIN // 128               # 2 din chunks
NPHASE = 4                    # B dither phases
F32 = mybir.dt.float32
F32R = mybir.dt.float32r
TB = SEQ * BS                 # flattened (t, b) = 4096
XRING = 4                     # xT ring slots (64 steps each)
JSTEPS = 64                   # steps covered by one xT ring block


LAST_RESULTS = None

# step-tail schedule: which engine copies which z columns, and the DVE
# emission order ("PT" marks where the 4 PE transposes are emitted).
import json as _json
SCHED = {
    "act": [],
    "dve_cp": [(0, 256), (256, 512)],
    "dve": ["c0", "c1", "PT", "m0", "m1"],
    "mm": [(0, 0), (1, 0), (2, 0), (3, 0), (0, 1), (1, 1), (2, 1), (3, 1)],
}
if os.environ.get("KERNEL_SCHED"):
    SCHED = _json.loads(os.environ["KERNEL_SCHED"])
    SCHED["act"] = [tuple(x) for x in SCHED["act"]]
    SCHED["dve_cp"] = [tuple(x) for x in SCHED["dve_cp"]]
    SCHED["mm"] = [tuple(x) for x in SCHED["mm"]]


def _register_modrelu():
    """Register the fused modrelu custom DVE op (idempotent).

    out = z + clamp(z*C0, C1, -C1)  with z = Src0 + Src1
    equals sign(z) * relu(|z| + bias) for
      C0 = bias >= 0 ? BIG : -1.0 ,  C1 = |bias|   (per-partition scalars).
    """
    name = "MODRELU_STEP_ANT"
    for op in dve_ops.OPS:
        if op.name == name:
            return op

    z = Src0 + Src1
    spec = Spec(
        body=z + maxx(minn(z * C0, C1), Zero - C1),
        reference=lambda in0, in1, s0, s1, imm2: (in0 + in1)
        + np.maximum(np.minimum((in0 + in1) * s0, s1), -s1),
    )
    shas = {}
    for ver in ("v3", "v4"):
        try:
            uops = lower(spec, ver=ver)
        except Exception:
            continue
        shas[ver] = DveOpSpec(name=name, uops=uops, rd1_en=True).sha(ver)
    op = dve_ops.DveOp(name, spec, subdim=False, uops_sha=shas)
    dve_ops.OPS.append(op)
    row = max(dve_ops._SUB_OPCODE_FOR_NAME.values()) + 1
    assert row < 0x20, "custom DVE opcode rows exhausted"
    dve_ops._SUB_OPCODE_FOR_NAME[name] = row
    dve_ops.CUSTOM_DVE_SPECS[name] = spec
    return op


MODRELU = _register_modrelu()

_NC_CACHE = None


def _build_nc():
    """Build the (SPMD-identical) Bass program for one core."""
    nc = bacc.Bacc()

    xtr_d = nc.dram_tensor("xtr", [DIN, TB], F32, kind="ExternalInput")
    tp_d = nc.dram_tensor("tp", [DIN, UNITS], F32, kind="ExternalInput")
    # bd: NPHASE dithered f32r roundings of B, stacked on rows
    bd_d = nc.dram_tensor("bd", [NPHASE * UNITS, UNITS], F32, kind="ExternalInput")
    h0s_d = nc.dram_tensor("h0s", [UNITS, BS], F32, kind="ExternalInput")
    mv_d = nc.dram_tensor("mv", [128, 2], F32, kind="ExternalInput")
    id8_d = nc.dram_tensor("id8", [BS, BS], F32, kind="ExternalInput")
    y_d = nc.dram_tensor("y", [NK, 128, TB], F32, kind="ExternalOutput")

    NJ = TB // 512            # 8 xT blocks of 512 (t,b) columns

    with TileContext(nc) as tc:
        with (
            tc.tile_pool(name="persist", bufs=1) as pp,
            tc.tile_pool(name="psum", bufs=1, space="PSUM") as psp,
        ):
            # ---- persistent SBUF tensors -------------------------------
            xtr_sb = [pp.tile([128, TB], F32R, tag=f"xtr{i}", name=f"xtr{i}")
                      for i in range(ND)]
            tp_sb = [pp.tile([128, UNITS], F32R, tag=f"tp{i}", name=f"tp{i}")
                     for i in range(ND)]
            bd_sb = [[pp.tile([128, UNITS], F32R, tag=f"bd{p}_{k}",
                              name=f"bd{p}_{k}") for k in range(NK)]
                     for p in range(NPHASE)]
            h0s_sb = pp.tile([128, NK * BS], F32R, tag="h0s", name="h0s")
            mv_sb = pp.tile([128, 2], F32, tag="mv", name="mv_sb")
            id8_sb = pp.tile([BS, BS], F32, tag="id8", name="id8")
            # xT ring: slot r holds chunks k at cols [512k, 512(k+1))
            xtt_sb = [pp.tile([128, NK * 512], F32, tag=f"xtt{r}",
                              name=f"xtt{r}") for r in range(XRING)]
            # state archive (also the output buffer), f32r:
            # ybuf[u', TB*k + 8t+b] = h_{t+1}[b, 128k+u']
            ybuf = pp.tile([128, NK * TB], F32R, tag="ybuf", name="ybuf")
            zb_pp = [pp.tile([BS, UNITS], F32, tag=f"zb{i}", name=f"zb{i}")
                     for i in range(2)]

            # PSUM: z banks (batch-major matmul target), zt (unit-major,
            # transpose-mode target), phase-1 banks
            # per-half z tiles: the half-A copy must not wait on half-B's
            # matmuls (PSUM reader deps are per-tile). Single-buffered; the
            # next step's first writer WARs on this step's copy, long done.
            z_h = [psp.tile([BS, UNITS // 2], F32, tag=f"z{h}", name=f"z{h}")
                   for h in range(2)]
            # one zt tile per chunk-PAIR: mod_{01} gates on PTs g0+g1 only
            # (PSUM reader deps are tracked per-tile, not per-region).
            # Single-buffered: PT(t+1) WARs on the pair-mod(t), long done.
            zt_pr = [psp.tile([128, 2 * BS], F32, tag=f"zt{q}", name=f"zt{q}")
                     for q in range(2)]
            pre_ps = [psp.tile([128, 512], F32, tag=f"pre{i}", name=f"pre{i}")
                      for i in range(2)]

            mv_v = [mv_sb[:, q:q + 1] for q in range(2)]

            # ---- input DMAs (single queue -> single DMA semaphore) -----
            # Order by first use: phase-1 inputs, then the small step-0
            # tiles, then the B phases in step order (phase p gates step p).
            for i in range(ND):
                nc.sync.dma_start(out=xtr_sb[i][:],
                                  in_=xtr_d[128 * i:128 * (i + 1), :].bitcast(F32R))
                nc.sync.dma_start(out=tp_sb[i][:],
                                  in_=tp_d[128 * i:128 * (i + 1), :].bitcast(F32R))
            for k in range(NK):
                nc.sync.dma_start(out=h0s_sb[:, BS * k:BS * (k + 1)],
                                  in_=h0s_d[128 * k:128 * (k + 1), :].bitcast(F32R))
            nc.sync.dma_start(out=mv_sb[:], in_=mv_d[0:128, 0:2])
            nc.sync.dma_start(out=id8_sb[:], in_=id8_d[:])
            for p in range(NPHASE):
                for k in range(NK):
                    r0 = UNITS * p + 128 * k
                    nc.sync.dma_start(out=bd_sb[p][k][:],
                                      in_=bd_d[r0:r0 + 128, :].bitcast(F32R))

            def emit_phase1(j, m):
                """xT block j, chunk m -> xtt_sb[j % XRING][:, 512m:+512]."""
                ps = pre_ps[(j * NK + m) % 2]
                for i in range(ND):
                    nc.tensor.matmul(
                        ps[:],
                        tp_sb[i][:, 128 * m:128 * (m + 1)],
                        xtr_sb[i][:, 512 * j:512 * (j + 1)],
                        start=(i == 0),
                        stop=(i == ND - 1),
                    )
                nc.scalar.copy(
                    xtt_sb[j % XRING][:, 512 * m:512 * (m + 1)], ps[:])

            # prologue: first 3 ring blocks
            for j in range(min(3, NJ)):
                for m in range(NK):
                    emit_phase1(j, m)

            # ---- the 512-step recurrence -------------------------------
            for t in range(SEQ):
                # paced phase-1 emission: one chunk per 16 steps
                if t % 16 == 0:
                    j = t // JSTEPS + 3
                    m = (t // 16) % NK
                    if j < NJ:
                        emit_phase1(j, m)

                p = t % 2
                ph = t % NPHASE
                zb = zb_pp[p]
                xtt = xtt_sb[(t // JSTEPS) % XRING]
                xc0 = 512 * 0 + BS * (t % JSTEPS)   # col offset within chunk 0

                def state_ap(k):
                    return (h0s_sb[:, BS * k:BS * (k + 1)] if t == 0
                            else ybuf[:, TB * k + BS * (t - 1):TB * k + BS * t])

                def mm(k, h, start, stop):
                    cols = slice(256 * h, 256 * (h + 1))
                    nc.tensor.matmul(z_h[h][:], state_ap(k),
                                     bd_sb[ph][k][:, cols],
                                     start=start, stop=stop)

                def pt(g):
                    nc.tensor.transpose(
                        zt_pr[g // 2][:, BS * (g % 2):BS * (g % 2 + 1)],
                        zb[:, 128 * g:128 * (g + 1)],
                        id8_sb[:])

                yb_v = ybuf[:].rearrange("p (k c) -> p k c", k=NK)
                xt_v = xtt[:].rearrange("p (k c) -> p k c", k=NK)

                def mod(q):
                    # fused modrelu for chunk pair (2q, 2q+1) in one op
                    nc.vector._custom_dve(
                        MODRELU,
                        out=yb_v[:, 2 * q:2 * q + 2, BS * t:BS * (t + 1)],
                        in0=zt_pr[q][:],
                        in1=xt_v[:, 2 * q:2 * q + 2,
                                 BS * (t % JSTEPS):BS * (t % JSTEPS) + BS],
                        s0=mv_v[q],
                        s1=1.0,
                    )

                def cp(eng, c0, c1):
                    h = c0 // 256
                    (nc.scalar.copy if eng == "act" else
                     nc.vector.tensor_copy)(
                        zb[:, c0:c1], z_h[h][:, c0 - 256 * h:c1 - 256 * h])

                # half A (cols 0:256, chunks 0/1) finalizes after 4 MMs so
                # its tail overlaps half B's MMs; chunk gates of step t+1
                # then consume chunks in production order. SCHED controls
                # the MM issue order, copy engines, and the DVE FIFO order.
                seen = {0: 0, 1: 0}
                for k, h in SCHED["mm"]:
                    mm(k, h, start=(seen[h] == 0), stop=(seen[h] == NK - 1))
                    seen[h] += 1
                for tok in SCHED["act"]:
                    cp("act", *tok)
                dve_toks = {f"c{i}": ("cp",) + tok
                            for i, tok in enumerate(SCHED["dve_cp"])}
                done_pt = set()
                for tok in SCHED["dve"]:
                    if tok == "PT":
                        for g in range(NK):
                            if g not in done_pt:
                                done_pt.add(g)
                                pt(g)
                    elif tok.startswith("p"):
                        g = int(tok[1])
                        if g not in done_pt:
                            done_pt.add(g)
                            pt(g)
                    elif tok.startswith("m"):
                        mod(int(tok[1]))
                    elif tok in dve_toks:
                        cp("dve", *dve_toks[tok][1:])
                for g in range(NK):
                    if g not in done_pt:
                        pt(g)

                # stream finished state blocks out to HBM
                if (t + 1) % JSTEPS == 0:
                    blk = (t + 1) // JSTEPS - 1
                    lo, hi = 512 * blk, 512 * (blk + 1)
                    for k in range(NK):
                        nc.sync.dma_start(
                            out=y_d[k, :, lo:hi],
                            in_=ybuf[:, TB * k + lo:TB * k + hi].bitcast(F32),
                        )

    return nc


def _get_nc():
    global _NC_CACHE
    if _NC_CACHE is None:
        nc = _build_nc()
        nc.finalize()          # run the bacc lowering passes
        _NC_CACHE = nc
    return _NC_CACHE


def _round_f32r(a):
    """Round fp32 values to the f32r format: round-to-nearest-even to 11
    explicit mantissa bits (low 12 bits zero) — HW-verified against the DVE
    f32->f32r rounding copy. Raw fp32 bits fed to an f32r matmul corrupt it."""
    u = np.ascontiguousarray(a, dtype=np.float32).view(np.uint32).copy()
    u += np.uint32(0x7FF) + ((u >> np.uint32(12)) & np.uint32(1))
    u &= np.uint32(0xFFFFF000)
    return u.view(np.float32)


def _build_perm(bias):
    """Assign units to permuted columns (chunk-major) so each partition's
    chunk-PAIR slots {(2q, p), (2q+1, p)} hold same-class units (class =
    sign of bias), enabling one fused modrelu per pair with a per-partition
    m scalar. Returns (cols, mvec, biasr): cols[j] = original unit at
    permuted column j; mvec[q, p] = m for pair q partition p; biasr = bias
    with at most one parity-fix reclassification (error <= 2|b| per step,
    applied to the smallest-|b| positive unit)."""
    U = bias.shape[0]
    cls = np.where(bias >= 0, 1, 0)
    cls[np.abs(bias) < 1e-12] = 1
    biasr = bias.astype(np.float64).copy()
    NP = int(cls.sum())
    if NP % 2 == 1:
        pos_idx = np.where(cls == 1)[0]
        u = pos_idx[np.argmin(np.abs(bias[pos_idx]))]
        cls[u] = 0
        biasr[u] = -abs(biasr[u]) if biasr[u] != 0 else -1e-12
        NP -= 1
    lo, hi = max(0, NP - U // 2), min(U // 2, NP)
    a = min(max(lo, (NP // 2) & ~1), hi & ~1)
    pos = list(np.where(cls == 1)[0])
    neg = list(np.where(cls == 0)[0])
    cols = np.empty(U, dtype=np.int64)
    mvec = np.empty((2, 128), dtype=np.float32)
    for half, npos in ((0, a), (1, NP - a)):
        slots = [pos.pop(0) for _ in range(npos)]
        slots += [neg.pop(0) for _ in range(256 - npos)]
        for p in range(128):
            cols[128 * (2 * half) + p] = slots[2 * p]
            cols[128 * (2 * half + 1) + p] = slots[2 * p + 1]
            mvec[half, p] = np.float32(1e20 if 2 * p < npos else -1.0)
    return cols, mvec, biasr


def _pack_inputs(x, T, B, bias, h0):
    """Build the per-core input maps (D-scaled, class-permuted)."""
    cols, mvec, biasr = _build_perm(bias)
    D = np.maximum(np.abs(biasr), 1e-12)[cols]            # per permuted col

    # B'' = D B D^-1 in permuted coords; 4-phase dithered f32r roundings:
    # partial sums track k*B so per-step rounding error mostly cancels.
    Bss = D[:, None] * B.astype(np.float64)[np.ix_(cols, cols)] / D[None, :]
    b1 = _round_f32r(Bss.astype(np.float32))
    bs = [b1]
    acc = b1.astype(np.float64)
    for k in range(2, NPHASE + 1):
        bk = _round_f32r((k * Bss - acc).astype(np.float32))
        bs.append(bk)
        acc += bk
    bd = np.concatenate(bs, axis=0)                       # [4*U, U]

    h0p = (h0.astype(np.float64)[cols] / D).astype(np.float32)
    h0s = _round_f32r(np.repeat(h0p[:, None], BS, axis=1))

    base = {
        "tp": _round_f32r((T.astype(np.float64)[:, cols] / D[None, :])
                          .astype(np.float32)),
        "bd": bd,
        "h0s": np.ascontiguousarray(h0s),
        "mv": np.ascontiguousarray(mvec.T),               # [128, 2]
        "id8": np.eye(BS, dtype=np.float32),
    }
    maps = []
    for c in range(NCORES):
        xs = x[c * BS:(c + 1) * BS]                       # [BS, SEQ, DIN]
        xtr = np.ascontiguousarray(
            xs.transpose(2, 1, 0).reshape(DIN, TB))       # [DIN, (t, b)]
        m = dict(base)
        m["xtr"] = _round_f32r(xtr)
        maps.append(m)
    inv = np.empty(UNITS, dtype=np.int64)
    inv[cols] = np.arange(UNITS)
    return maps, D.astype(np.float32), inv


def kernel(x, T, B, bias, h0):
    """Full-input, full-output entry point."""
    global LAST_RESULTS
    x = np.ascontiguousarray(np.asarray(x, dtype=np.float32))
    T = np.ascontiguousarray(np.asarray(T, dtype=np.float32))
    B = np.ascontiguousarray(np.asarray(B, dtype=np.float32))
    bias = np.asarray(bias, dtype=np.float32)
    h0 = np.asarray(h0, dtype=np.float32)

    in_maps, Dscale, inv = _pack_inputs(x, T, B, bias, h0)

    nc = _get_nc()
    trace = bool(int(os.environ.get("KERNEL_TRACE", "0")))
    res = bass_utils.run_bass_kernel_spmd(
        nc, in_maps, list(range(NCORES)), trace=trace)
    LAST_RESULTS = res

    out = np.empty((SEQ, BATCH, UNITS), dtype=np.float32)
    for c in range(NCORES):
        y = res.results[c]["y"].reshape(NK, 128, SEQ, BS)
        # permuted col j at [t, b]: y[j//128, j%128, t, b]; un-scale by D
        # and un-permute (out unit u = permuted col inv[u])
        yp = (y.transpose(2, 3, 0, 1).reshape(SEQ, BS, UNITS)
              * Dscale[None, None, :])
        out[:, c * BS:(c + 1) * BS, :] = yp[:, :, inv]
    return out


if __name__ == "__main__":
    rng = np.random.default_rng(0)
    x = rng.standard_normal((BATCH, SEQ, DIN), dtype=np.float32)
    T = rng.standard_normal((DIN, UNITS), dtype=np.float32) / DIN
    B = rng.standard_normal((UNITS, UNITS), dtype=np.float32) / 22.0
    bias = rng.uniform(-0.01, 0.01, UNITS).astype(np.float32)
    h0 = np.zeros(UNITS, dtype=np.float32)
    out = kernel(x=x, T=T, B=B, bias=bias, h0=h0)
    print("out", out.shape, out.dtype, float(np.abs(out).mean()))
